# revision 2
# baseline (speedup 1.0000x reference)
"""Trainium2 Bass kernel for the retrieval-KNN module:

    h   = y @ Wy_w.T + Wy_b                      # [B,N,1024]
    dz  = dic_z @ Wz_w.T + Wz_b                  # [K,1024]
    att = softmax(h @ dz.T / sqrt(1024))         # [B,N,K]
    z   = einsum('bnk,k,ke->bne', att, prior, dic_z)

Strategy: data-parallel over B across 8 NeuronCores (8 batches = 2048
tokens per core); dic_z / weights replicated. All matmuls in bf16 (PE
fp32 runs at 1/4 rate), fp32 PSUM accumulation, transposes done by the
DMA transpose xbar on bf16 staged copies.  softmax has no max-subtraction
(logits are O(+-5) for this distribution) and folds the prior in as an
exp() bias: Ep = exp(logits/32 + log(prior)); then
z = (Ep.T @ dic_z) / (Ep.T @ (1/prior)) per token.
"""

import sys

import numpy as np


def _ensure_paths():
    for p in ("/opt/trn_rl_repo",):
        if p not in sys.path:
            sys.path.append(p)


_ensure_paths()

from contextlib import ExitStack  # noqa: E402

import concourse.bacc as bacc  # noqa: E402
import concourse.mybir as mybir  # noqa: E402
import concourse.tile as tile  # noqa: E402
from concourse import bass_utils  # noqa: E402
from concourse.bass import ts  # noqa: E402

F32 = mybir.dt.float32
BF16 = mybir.dt.bfloat16
AF = mybir.ActivationFunctionType

NCORES = 8
# Full problem dims (hardcoded per spec nn_Causal_v_69054484185473)
B, N, EMB = 64, 256, 1024
FULL = dict(T=(B // NCORES) * N, I=1024, O=1024, J=2048, K=4096,
            TC=512, KG=512, EC=512)
SCALE = 1.0 / 32.0  # 1/sqrt(EMB)


def build_bass(T=2048, I=1024, O=1024, J=2048, K=4096, TC=512, KG=512,
               EC=512, dt_mm=BF16, scale=SCALE, num_devices=NCORES):
    """Build the per-core Bass program (SPMD: same NEFF on every core)."""
    IC, OC, JC, KC, TS = I // 128, O // 128, J // 128, K // 128, TC // 128
    NTC, NKG, NEC = T // TC, K // KG, J // EC

    nc = bacc.Bacc("TRN2", target_bir_lowering=False, debug=False,
                   num_devices=num_devices)
    y = nc.dram_tensor("y", [T, I], F32, kind="ExternalInput").ap()
    Wy_w = nc.dram_tensor("Wy_w", [O, I], F32, kind="ExternalInput").ap()
    Wy_b = nc.dram_tensor("Wy_b", [O], F32, kind="ExternalInput").ap()
    Wz_w = nc.dram_tensor("Wz_w", [O, J], F32, kind="ExternalInput").ap()
    Wz_b = nc.dram_tensor("Wz_b", [O], F32, kind="ExternalInput").ap()
    dic_z = nc.dram_tensor("dic_z", [K, J], F32, kind="ExternalInput").ap()
    logp_in = nc.dram_tensor("logp_in", [K], F32, kind="ExternalInput").ap()
    invp_in = nc.dram_tensor("invp_in", [K], F32, kind="ExternalInput").ap()
    z = nc.dram_tensor("z", [T, J], F32, kind="ExternalOutput").ap()

    with tile.TileContext(nc) as tc, ExitStack() as stack:
        drp = stack.enter_context(tc.tile_pool(name="dram", bufs=1, space="DRAM"))
        y_d = drp.tile([T, I], dt_mm)
        wy_d = drp.tile([O, I], dt_mm)
        wz_d = drp.tile([O, J], dt_mm)
        dic_d = drp.tile([K, J], dt_mm)
        dzT_d = drp.tile([O, K], dt_mm)

        const = stack.enter_context(tc.tile_pool(name="const", bufs=1))
        logp = const.tile([128, KC], F32)
        nc.sync.dma_start(logp[:], logp_in.rearrange("(c p) -> p c", p=128))
        invp_f = const.tile([128, KC], F32)
        nc.sync.dma_start(invp_f[:], invp_in.rearrange("(c p) -> p c", p=128))
        invp = const.tile([128, KC], dt_mm)
        nc.vector.tensor_copy(invp[:], invp_f[:])
        wyb = const.tile([128, OC], F32)
        nc.sync.dma_start(wyb[:], Wy_b.rearrange("(c p) -> p c", p=128))
        wzb = const.tile([128, OC], F32)
        nc.sync.dma_start(wzb[:], Wz_b.rearrange("(c p) -> p c", p=128))

        # ---- stage bf16 copies of y / Wy / Wz / dic_z in DRAM (SWDGE casts)
        with tc.tile_pool(name="cast", bufs=4) as cast:
            for (src, dst, rows, cols) in ((y, y_d, T, I), (Wy_w, wy_d, O, I),
                                           (Wz_w, wz_d, O, J),
                                           (dic_z, dic_d, K, J)):
                for r in range(rows // 128):
                    ct = cast.tile([128, cols], dt_mm, tag="ct", name="ct")
                    nc.gpsimd.dma_start(ct[:, :cols], src[ts(r, 128), :])
                    nc.sync.dma_start(dst[ts(r, 128), :], ct[:, :cols])

        # wyT[p, ic, o] = Wy_w[o, ic*128+p]  (lhsT tiles for hT)
        wyT = const.tile([128, IC, O], dt_mm)
        for ic in range(IC):
            nc.sync.dma_start(wyT[:, ic, :], wy_d[:, ts(ic, 128)], transpose=True)

        # ---- phase dz: dzT_d[o, k] = (dic_z @ Wz_w.T + Wz_b).T in bf16
        with tc.tile_pool(name="wzt", bufs=1) as wztp, \
             tc.tile_pool(name="dzw", bufs=2) as dzw, \
             tc.tile_pool(name="dzp", bufs=2, space="PSUM") as dzp:
            wzT = wztp.tile([128, JC, O], dt_mm)
            for jc in range(JC):
                nc.sync.dma_start(wzT[:, jc, :], wz_d[:, ts(jc, 128)],
                                  transpose=True)
            for kg in range(NKG):
                dicT = dzw.tile([128, JC, KG], dt_mm, tag="dicT")
                for jc in range(JC):
                    nc.sync.dma_start(dicT[:, jc, :],
                                      dic_d[ts(kg, KG), ts(jc, 128)],
                                      transpose=True)
                for oc in range(OC):
                    ps = dzp.tile([128, KG], F32, tag="dzps")
                    for jc in range(JC):
                        nc.tensor.matmul(ps[:], wzT[:, jc, ts(oc, 128)],
                                         dicT[:, jc, :],
                                         start=(jc == 0), stop=(jc == JC - 1))
                    so = dzw.tile([128, KG], dt_mm, tag="dzso")
                    nc.scalar.activation(so[:], ps[:], AF.Identity,
                                         bias=wzb[:, oc:oc + 1])
                    nc.sync.dma_start(dzT_d[ts(oc, 128), ts(kg, KG)], so[:])

        # ---- main per-token-chunk pipeline
        mp = stack.enter_context(tc.tile_pool(name="mp", bufs=2))
        epp = stack.enter_context(tc.tile_pool(name="epp", bufs=1))
        zp = stack.enter_context(tc.tile_pool(name="zp", bufs=3))
        mps = stack.enter_context(tc.tile_pool(name="mps", bufs=2, space="PSUM"))
        spsp = stack.enter_context(tc.tile_pool(name="spsp", bufs=2, space="PSUM"))

        for tci in range(NTC):
            # yT[p, ic, t] = y[tci*TC + t, ic*128+p]
            yT = mp.tile([128, IC, TC], dt_mm, tag="yT")
            for ic in range(IC):
                nc.sync.dma_start(yT[:, ic, :], y_d[ts(tci, TC), ts(ic, 128)],
                                  transpose=True)
            # hT[p, oc, t] = h[t, oc*128+p]
            hT = mp.tile([128, OC, TC], dt_mm, tag="hT")
            for oc in range(OC):
                ps = mps.tile([128, TC], F32, tag="mm", name="ps")
                for ic in range(IC):
                    nc.tensor.matmul(ps[:], wyT[:, ic, ts(oc, 128)],
                                     yT[:, ic, :],
                                     start=(ic == 0), stop=(ic == IC - 1))
                nc.scalar.activation(hT[:, oc, :], ps[:], AF.Identity,
                                     bias=wyb[:, oc:oc + 1])
            # Ep[p, kc, t] = exp(logits[kc*128+p, t]*scale + log prior)
            Ep = epp.tile([128, KC, TC], dt_mm, tag="Ep")
            for kc in range(KC):
                dzTk = mp.tile([128, OC, 128], dt_mm, tag="dzTk", bufs=3)
                nc.sync.dma_start(
                    dzTk[:],
                    dzT_d[:, ts(kc, 128)].rearrange("(c p) m -> p c m", p=128))
                ps = mps.tile([128, TC], F32, tag="mm", name="ps")
                for oc in range(OC):
                    nc.tensor.matmul(ps[:], dzTk[:, oc, :], hT[:, oc, :],
                                     start=(oc == 0), stop=(oc == OC - 1))
                nc.scalar.activation(Ep[:, kc, :], ps[:], AF.Exp,
                                     bias=logp[:, kc:kc + 1], scale=scale)
            # weighted sum + softmax denominator
            rsum = mp.tile([128, TS], F32, tag="rsum")
            for ec in range(NEC):
                dicE = mp.tile([128, KC, EC], dt_mm, tag="dicE")
                nc.sync.dma_start(
                    dicE[:],
                    dic_d[:, ts(ec, EC)].rearrange("(c p) e -> p c e", p=128))
                for tsi in range(TS):
                    zps = mps.tile([128, EC], F32, tag="zps", name="zps")
                    if ec == 0:
                        sps = spsp.tile([128, 1], F32, tag="sps", name="sps")
                    for kc in range(KC):
                        nc.tensor.matmul(zps[:], Ep[:, kc, ts(tsi, 128)],
                                         dicE[:, kc, :],
                                         start=(kc == 0), stop=(kc == KC - 1))
                        if ec == 0:
                            nc.tensor.matmul(sps[:], Ep[:, kc, ts(tsi, 128)],
                                             invp[:, kc:kc + 1],
                                             start=(kc == 0),
                                             stop=(kc == KC - 1))
                    if ec == 0:
                        nc.vector.reciprocal(rsum[:, tsi:tsi + 1], sps[:])
                    zt = zp.tile([128, EC], F32, tag="zt", name="zt")
                    nc.vector.tensor_scalar_mul(zt[:], zps[:],
                                                rsum[:, tsi:tsi + 1])
                    row0 = tci * TC + tsi * 128
                    nc.sync.dma_start(z[row0:row0 + 128, ts(ec, EC)], zt[:])

    nc.compile()
    return nc


_NC_CACHE = {}


def _get_nc():
    key = "full"
    if key not in _NC_CACHE:
        _NC_CACHE[key] = build_bass(**FULL)
    return _NC_CACHE[key]


def make_in_maps(y, Wy_w, Wy_b, Wz_w, Wz_b, dic_z, prior):
    Bs = B // NCORES
    prior = np.asarray(prior, np.float32)
    shared = {
        "Wy_w": np.ascontiguousarray(np.asarray(Wy_w, np.float32)),
        "Wy_b": np.ascontiguousarray(np.asarray(Wy_b, np.float32)),
        "Wz_w": np.ascontiguousarray(np.asarray(Wz_w, np.float32)),
        "Wz_b": np.ascontiguousarray(np.asarray(Wz_b, np.float32)),
        "dic_z": np.ascontiguousarray(np.asarray(dic_z, np.float32)),
        "logp_in": np.log(prior).astype(np.float32),
        "invp_in": (1.0 / prior).astype(np.float32),
    }
    y = np.asarray(y, np.float32)
    return [{**shared,
             "y": np.ascontiguousarray(y[i * Bs:(i + 1) * Bs].reshape(Bs * N, EMB))}
            for i in range(NCORES)]


def run_spmd(in_maps, **kw):
    nc = _get_nc()
    res = bass_utils.run_bass_kernel_spmd(nc, in_maps,
                                          core_ids=list(range(NCORES)), **kw)
    Bs = B // NCORES
    z = np.concatenate(
        [res.results[i]["z"].reshape(Bs, N, 2048) for i in range(NCORES)],
        axis=0)
    return z.astype(np.float32), res


def kernel(y, Wy_w, Wy_b, Wz_w, Wz_b, dic_z, prior):
    """Full-input / full-output entry point (shards over B internally)."""
    z, _ = run_spmd(make_in_maps(y, Wy_w, Wy_b, Wz_w, Wz_b, dic_z, prior))
    return z
